# revision 24
# baseline (speedup 1.0000x reference)
"""Local (windowed) attention kernel for TRN2, 8 NeuronCores, SPMD. v2.

Reference computation (B=4, N=8192, DIM=1024, H=16, DH=64, W=128):
    q = x @ wq ; k,v = split(x @ wkv)
    per (batch, head, window of 128): attend to [prev window, cur window]
    with causal mask (j > i + W masked), softmax, out = attn @ v
    out = out @ wo + bo

Sharding: sequence dim split into 8 contiguous chunks of 1024 tokens, one
per core, each with a 128-token front halo (zeros for core 0 — matches the
reference's zero-pad).  Weights replicated; no collectives.

v2 design (vs the transpose-heavy v1):
  * scores are computed TRANSPOSED on PE: sT[j, i] = k_j . q_i per
    (head, key-window), serving two query-windows per matmul (N=256).
    The probs then feed attn@v directly as the moving operand — the
    1024 per-batch PE transposes and per-head scale-copies of v1 are gone.
  * causal mask on the cur-block via one additive matmul (maskU^T @ id).
  * softmax denominators: column sums of exp tiles via M=1 matmuls with a
    stationary ones-vector (2 per head-window, accumulated over the two
    key-windows), packed 4 rows per PSUM bank at partitions 0/32/64/96.
  * normalize fused into PSUM evacuation: DVE reciprocal of the sums row,
    GpSimd partition_broadcast to 64 rows, then one DVE tensor_mul per
    (head-parity, 4-head group) writing normalized bf16 straight into the
    feature-major attention output aoT.
  * output DRAM tensor is bf16 (host converts to f32) to halve store DMA.
"""

import numpy as np
import ml_dtypes

import concourse.bass as bass
import concourse.bacc as bacc
import concourse.mybir as mybir
import concourse.tile as tile
from concourse.bass_utils import run_bass_kernel_spmd

B, N, DIM = 4, 8192, 1024
H, DH, W = 16, 64, 128
NCORES = 8
TOW = N // NCORES          # own tokens per core per batch   = 1024
TH = TOW + W               # with front halo                 = 1152
NW = TOW // W              # query windows per core-batch    = 8
NKW = NW + 1               # key windows incl. halo          = 9
KT = DIM // 128            # contraction tiles               = 8
MT = DIM // 128            # inner/output tiles              = 8
SCALE = DH ** -0.5

BF16 = mybir.dt.bfloat16
F32 = mybir.dt.float32
AF = mybir.ActivationFunctionType

TRACE = False              # set by test.py to collect an NTFF profile
TRACE_KW = {}
LAST_RESULT = None         # BassKernelResults stash when TRACE
REPEAT = 1                 # whole-computation repeats inside the NEFF (bench)


def _build_bass():
    nc = bacc.Bacc(None, target_bir_lowering=False)
    xT = nc.declare_dram_parameter("xT", [B, DIM, TH], BF16, isOutput=False)
    wq = nc.declare_dram_parameter("wq", [DIM, DIM], BF16, isOutput=False)
    wkv = nc.declare_dram_parameter("wkv", [DIM, 2 * DIM], BF16, isOutput=False)
    wo = nc.declare_dram_parameter("wo", [DIM, DIM], BF16, isOutput=False)
    bo_pm = nc.declare_dram_parameter("bo_pm", [128, MT], F32, isOutput=False)
    maskU = nc.declare_dram_parameter("maskU", [128, 128], BF16, isOutput=False)
    outT = nc.declare_dram_parameter("outT", [B, DIM, TOW], BF16, isOutput=True)

    with tile.TileContext(nc) as tc:
        with (
            tc.tile_pool(name="wpool", bufs=1) as wpool,
            tc.tile_pool(name="xpool", bufs=2) as xpool,
            tc.tile_pool(name="actpool", bufs=1) as actpool,
            tc.tile_pool(name="apool", bufs=3) as apool,
            tc.tile_pool(name="rpool", bufs=2) as rpool,
            tc.tile_pool(name="bpool", bufs=3) as bpool,
            tc.tile_pool(name="opool", bufs=1) as opool,
            tc.tile_pool(name="pbig", bufs=4, space="PSUM") as pbig,
            tc.tile_pool(name="ppv", bufs=2, space="PSUM") as ppv,
            tc.tile_pool(name="prows", bufs=1, space="PSUM") as prows,
        ):
            # ---- replicated constants (loaded once) ----
            wq_sb = wpool.tile([128, KT, DIM], BF16)
            wkv_sb = wpool.tile([128, KT, 2 * DIM], BF16)
            wo_sb = wpool.tile([128, KT, DIM], BF16)
            bo_sb = wpool.tile([128, MT], F32)
            mask_sb = wpool.tile([128, 2, 128], BF16)
            ones_sb = wpool.tile([128, 1], BF16)
            for k in range(KT):
                nc.sync.dma_start(out=wq_sb[:, k, :], in_=wq[k * 128:(k + 1) * 128, :])
                nc.sync.dma_start(out=wkv_sb[:, k, :], in_=wkv[k * 128:(k + 1) * 128, :])
                nc.sync.dma_start(out=wo_sb[:, k, :], in_=wo[k * 128:(k + 1) * 128, :])
            nc.sync.dma_start(out=bo_sb, in_=bo_pm[:])
            nc.sync.dma_start(out=mask_sb[:, 0, :], in_=maskU[:])
            nc.sync.dma_start(out=mask_sb[:, 1, :], in_=maskU[:])
            nc.vector.memset(ones_sb, 1.0)

            for b in [bb % B for bb in range(B * REPEAT)]:
                # ---- load xT slice (feature-major, with halo) ----
                x_sb = xpool.tile([128, KT, TH], BF16, tag="x")
                nc.gpsimd.dma_start(
                    out=x_sb[:],
                    in_=xT[b].rearrange("(k p) t -> p k t", p=128),
                )

                qT = actpool.tile([128, MT, TOW], BF16, tag="qT")
                kTt = actpool.tile([128, MT, TH], BF16, tag="kT")
                v_sb = actpool.tile([128, NKW, DIM], BF16, tag="v")
                aoT = actpool.tile([128, MT, TOW], BF16, tag="aoT")

                # ---- q projection, feature-major: qT[m] = wq[:,m].T @ x ----
                for m in range(MT):
                    for c in range(2):
                        ps = pbig.tile([128, 512], F32, tag="big")
                        for k in range(KT):
                            nc.tensor.matmul(
                                ps,
                                lhsT=wq_sb[:, k, m * 128:(m + 1) * 128],
                                rhs=x_sb[:, k, W + c * 512:W + (c + 1) * 512],
                                start=(k == 0),
                                stop=(k == KT - 1),
                            )
                        nc.vector.tensor_copy(
                            out=qT[:, m, c * 512:(c + 1) * 512], in_=ps
                        )

                # ---- k projection, feature-major (incl. halo) ----
                for m in range(MT):
                    for c in range(3):
                        ps = pbig.tile([128, 512], F32, tag="big")
                        for k in range(KT):
                            nc.tensor.matmul(
                                ps[:, 0:384],
                                lhsT=wkv_sb[:, k, m * 128:(m + 1) * 128],
                                rhs=x_sb[:, k, c * 384:(c + 1) * 384],
                                start=(k == 0),
                                stop=(k == KT - 1),
                            )
                        nc.scalar.activation(
                            out=kTt[:, m, c * 384:(c + 1) * 384],
                            in_=ps[:, 0:384],
                            func=AF.Copy,
                            bias=0.0,
                            scale=1.0,
                        )

                # ---- v projection, token-major per key-window ----
                for kw in range(NKW):
                    for c in range(2):
                        ps = pbig.tile([128, 512], F32, tag="big")
                        for k in range(KT):
                            nc.tensor.matmul(
                                ps,
                                lhsT=x_sb[:, k, kw * 128:(kw + 1) * 128],
                                rhs=wkv_sb[:, k, DIM + c * 512:DIM + (c + 1) * 512],
                                start=(k == 0),
                                stop=(k == KT - 1),
                            )
                        nc.scalar.activation(
                            out=v_sb[:, kw, c * 512:(c + 1) * 512],
                            in_=ps,
                            func=AF.Copy,
                            bias=0.0,
                            scale=1.0,
                        )

                # ---- attention: key-window-major transposed scores ----
                def _emit_out_proj(c, _b=b, _qT=qT, _aoT=aoT):
                    for m in range(MT):
                        ps = pbig.tile([128, 512], F32, tag="big", name="ps_op")
                        for k in range(KT):
                            nc.tensor.matmul(
                                ps,
                                lhsT=wo_sb[:, k, m * 128:(m + 1) * 128],
                                rhs=_aoT[:, k, c * 512:(c + 1) * 512],
                                start=(k == 0),
                                stop=(k == KT - 1),
                            )
                        osb = opool.tile([128, 512], BF16, tag="outsb")
                        nc.vector.tensor_scalar_add(
                            out=osb, in0=ps, scalar1=bo_sb[:, m:m + 1]
                        )
                        nc.sync.dma_start(
                            out=outT[_b, m * 128:(m + 1) * 128,
                                     c * 512:(c + 1) * 512],
                            in_=osb,
                        )

                attn_tiles = []
                for kw in range(NKW):
                    has_a = kw >= 1          # cur-block for qw = kw-1
                    has_b = kw <= NW - 1     # prev-block for qw = kw
                    ncols = 128 * (has_a + has_b)
                    # qT is indexed by own tokens (haloed minus W)
                    qcol0 = (kw - 1) * 128 if has_a else 0
                    at = apool.tile([128, H, 256], BF16, tag="attn")
                    attn_tiles.append(at)
                    # attn tile slot permutation: slot(h) = (h%2)*8 + h//2,
                    # so same-row-group head pairs (same r, adjacent m) are
                    # adjacent slots.  Same-row-group score MMs serialize in
                    # the PE array, making same-bank drains safe.
                    for e in range(2):
                        r = e * 64
                        for mp in range(0, MT, 2):
                            sc = pbig.tile([128, 512], F32, tag="big")
                            for mi in range(2):
                                nc.tensor.matmul(
                                    sc[:, mi * 256:mi * 256 + ncols],
                                    lhsT=kTt[r:r + 64, mp + mi,
                                             kw * 128:(kw + 1) * 128],
                                    rhs=qT[r:r + 64, mp + mi,
                                           qcol0:qcol0 + ncols],
                                    start=True,
                                    stop=True,
                                    skip_group_check=True,
                                )
                            sl = e * 8 + mp
                            nc.scalar.activation(
                                out=at[:, sl:sl + 2, 0:ncols],
                                in_=sc[:].rearrange("p (h i) -> p h i", i=256)[
                                    :, :, 0:ncols
                                ],
                                func=AF.Exp,
                                bias=0.0,
                                scale=SCALE,
                            )
                            if has_a:
                                # multiplicative causal mask on the cur block:
                                # at[j, i] *= (j <= i), zeroing masked probs
                                nc.vector.tensor_mul(
                                    out=at[:, sl:sl + 2, 0:128],
                                    in0=at[:, sl:sl + 2, 0:128],
                                    in1=mask_sb,
                                )

                    # ---- pv + sums + normalize for qw = kw-1 ----
                    if kw == 0:
                        continue
                    qw = kw - 1
                    at_prev = attn_tiles[qw]       # tile kw-1: B block
                    at_cur = attn_tiles[kw]        # tile kw:   A block
                    bcol = 0 if qw == 0 else 128
                    for c in range(2):
                        pvt = ppv.tile([128, 512], F32, tag="pv")
                        srow = prows.tile([1, 1024], F32, tag="srows")
                        pv_last = None
                        srow_last = None
                        for hh in range(8):
                            h = 8 * c + hh
                            s, p = hh // 2, hh % 2
                            sl = (h % 2) * 8 + h // 2
                            hv = slice(h * 64, (h + 1) * 64)
                            pv_out = pvt[p * 64:p * 64 + 64, s * 128:(s + 1) * 128]
                            nc.tensor.matmul(
                                pv_out,
                                lhsT=v_sb[:, qw, hv],
                                rhs=at_prev[:, sl, bcol:bcol + 128],
                                start=True,
                                stop=True,
                            )
                            pv_last = nc.tensor.matmul(
                                pv_out,
                                lhsT=v_sb[:, kw, hv],
                                rhs=at_cur[:, sl, 0:128],
                                start=False,
                                stop=False,
                                skip_group_check=True,
                            )
                        srow_lasts = []
                        for p in range(2):
                            slb = p * 8 + 4 * c
                            sum_out = srow[0:1, p * 512:(p + 1) * 512]
                            nc.tensor.matmul(
                                sum_out,
                                lhsT=ones_sb[:, 0:1],
                                rhs=at_prev[:, slb:slb + 4, bcol:bcol + 128],
                                start=True,
                                stop=True,
                                skip_group_check=True,
                            )
                            srow_lasts.append(nc.tensor.matmul(
                                sum_out,
                                lhsT=ones_sb[:, 0:1],
                                rhs=at_cur[:, slb:slb + 4, 0:128],
                                start=False,
                                stop=False,
                                skip_group_check=True,
                            ))
                        rr = rpool.tile([1, 1024], F32, tag="recip")
                        ri = nc.vector.reciprocal_approx_fast(
                            out=rr[0:1, :], in_=srow[0:1, :]
                        )
                        for sv in srow_lasts:
                            tile.add_dep_helper(ri.ins, sv.ins)
                        for p in range(2):
                            rt = bpool.tile([128, 512], F32, tag="bcast")
                            nc.gpsimd.partition_broadcast(
                                rt[0:128, :],
                                rr[0:1, p * 512:(p + 1) * 512],
                                channels=128,
                            )
                            mi = nc.vector.tensor_mul(
                                out=aoT[p * 64:p * 64 + 64, 4 * c:4 * c + 4,
                                        qw * 128:(qw + 1) * 128],
                                in0=pvt[p * 64:p * 64 + 64, :].rearrange(
                                    "p (s i) -> p s i", i=128
                                ),
                                in1=rt[p * 64:p * 64 + 64, :].rearrange(
                                    "p (s i) -> p s i", i=128
                                ),
                            )
                            tile.add_dep_helper(mi.ins, pv_last.ins)

                _emit_out_proj(0)
                _emit_out_proj(1)
    nc.compile()
    return nc


_NC_CACHE = None


def _get_nc():
    global _NC_CACHE
    if _NC_CACHE is None:
        _NC_CACHE = _build_bass()
    return _NC_CACHE


def kernel(x, wq, wkv, wo, bo):
    global LAST_RESULT
    bfd = ml_dtypes.bfloat16
    x = np.asarray(x, np.float32)
    wq_b = np.asarray(wq, np.float32).astype(bfd)
    wkv_b = np.asarray(wkv, np.float32).astype(bfd)
    wo_b = np.asarray(wo, np.float32).astype(bfd)
    bo_pm = np.ascontiguousarray(
        np.asarray(bo, np.float32).reshape(MT, 128).T
    )
    # maskU[j, i] = 0 where cur-window key j > query i (causal), else 1
    maskU = np.where(
        np.arange(W)[:, None] > np.arange(W)[None, :], 0.0, 1.0
    ).astype(bfd)

    xb = x.astype(bfd)
    in_maps = []
    for c in range(NCORES):
        lo, hi = c * TOW - W, (c + 1) * TOW
        if c == 0:
            sl = np.concatenate(
                [np.zeros((B, W, DIM), bfd), xb[:, :hi]], axis=1
            )
        else:
            sl = xb[:, lo:hi]
        xT_c = np.ascontiguousarray(sl.transpose(0, 2, 1))  # [B, DIM, TH]
        in_maps.append(
            dict(xT=xT_c, wq=wq_b, wkv=wkv_b, wo=wo_b, bo_pm=bo_pm,
                 maskU=maskU)
        )

    nc = _get_nc()
    res = run_bass_kernel_spmd(
        nc, in_maps, list(range(NCORES)), trace=TRACE, **TRACE_KW
    )
    if TRACE:
        LAST_RESULT = res
    out = np.empty((B, N, DIM), np.float32)
    for c in range(NCORES):
        out[:, c * TOW:(c + 1) * TOW, :] = (
            res.results[c]["outT"].astype(np.float32).transpose(0, 2, 1)
        )
    return out


# revision 26
# speedup vs baseline: 1.0174x; 1.0174x over previous
"""Local (windowed) attention kernel for TRN2, 8 NeuronCores, SPMD. v2.

Reference computation (B=4, N=8192, DIM=1024, H=16, DH=64, W=128):
    q = x @ wq ; k,v = split(x @ wkv)
    per (batch, head, window of 128): attend to [prev window, cur window]
    with causal mask (j > i + W masked), softmax, out = attn @ v
    out = out @ wo + bo

Sharding: sequence dim split into 8 contiguous chunks of 1024 tokens, one
per core, each with a 128-token front halo (zeros for core 0 — matches the
reference's zero-pad).  Weights replicated; no collectives.

v2 design (vs the transpose-heavy v1):
  * scores are computed TRANSPOSED on PE: sT[j, i] = k_j . q_i per
    (head, key-window), serving two query-windows per matmul (N=256).
    The probs then feed attn@v directly as the moving operand — the
    1024 per-batch PE transposes and per-head scale-copies of v1 are gone.
  * causal mask on the cur-block via one additive matmul (maskU^T @ id).
  * softmax denominators: column sums of exp tiles via M=1 matmuls with a
    stationary ones-vector (2 per head-window, accumulated over the two
    key-windows), packed 4 rows per PSUM bank at partitions 0/32/64/96.
  * normalize fused into PSUM evacuation: DVE reciprocal of the sums row,
    GpSimd partition_broadcast to 64 rows, then one DVE tensor_mul per
    (head-parity, 4-head group) writing normalized bf16 straight into the
    feature-major attention output aoT.
  * output DRAM tensor is bf16 (host converts to f32) to halve store DMA.
"""

import numpy as np
import ml_dtypes

import concourse.bass as bass
import concourse.bacc as bacc
import concourse.mybir as mybir
import concourse.tile as tile
from concourse.bass_utils import run_bass_kernel_spmd

B, N, DIM = 4, 8192, 1024
H, DH, W = 16, 64, 128
NCORES = 8
TOW = N // NCORES          # own tokens per core per batch   = 1024
TH = TOW + W               # with front halo                 = 1152
NW = TOW // W              # query windows per core-batch    = 8
NKW = NW + 1               # key windows incl. halo          = 9
KT = DIM // 128            # contraction tiles               = 8
MT = DIM // 128            # inner/output tiles              = 8
SCALE = DH ** -0.5

BF16 = mybir.dt.bfloat16
F32 = mybir.dt.float32
AF = mybir.ActivationFunctionType

TRACE = False              # set by test.py to collect an NTFF profile
TRACE_KW = {}
LAST_RESULT = None         # BassKernelResults stash when TRACE
REPEAT = 1                 # whole-computation repeats inside the NEFF (bench)


def _build_bass():
    nc = bacc.Bacc(None, target_bir_lowering=False)
    xT = nc.declare_dram_parameter("xT", [B, DIM, TH], BF16, isOutput=False)
    wq = nc.declare_dram_parameter("wq", [DIM, DIM], BF16, isOutput=False)
    wkv = nc.declare_dram_parameter("wkv", [DIM, 2 * DIM], BF16, isOutput=False)
    wo = nc.declare_dram_parameter("wo", [DIM, DIM], BF16, isOutput=False)
    bo_pm = nc.declare_dram_parameter("bo_pm", [128, MT], F32, isOutput=False)
    maskU = nc.declare_dram_parameter("maskU", [128, 128], BF16, isOutput=False)
    outT = nc.declare_dram_parameter("outT", [B, DIM, TOW], BF16, isOutput=True)

    with tile.TileContext(nc) as tc:
        with (
            tc.tile_pool(name="wpool", bufs=1) as wpool,
            tc.tile_pool(name="xpool", bufs=2) as xpool,
            tc.tile_pool(name="actpool", bufs=1) as actpool,
            tc.tile_pool(name="apool", bufs=3) as apool,
            tc.tile_pool(name="rpool", bufs=2) as rpool,
            tc.tile_pool(name="bpool", bufs=3) as bpool,
            tc.tile_pool(name="opool", bufs=1) as opool,
            tc.tile_pool(name="pbig", bufs=4, space="PSUM") as pbig,
            tc.tile_pool(name="ppv", bufs=2, space="PSUM") as ppv,
            tc.tile_pool(name="prows", bufs=1, space="PSUM") as prows,
        ):
            # ---- replicated constants (loaded once) ----
            wq_sb = wpool.tile([128, KT, DIM], BF16)
            wkv_sb = wpool.tile([128, KT, 2 * DIM], BF16)
            wo_sb = wpool.tile([128, KT, DIM], BF16)
            bo_sb = wpool.tile([128, MT], F32)
            mask_sb = wpool.tile([128, 2, 128], BF16)
            ones_sb = wpool.tile([128, 1], BF16)
            for k in range(KT):
                nc.sync.dma_start(out=wq_sb[:, k, :], in_=wq[k * 128:(k + 1) * 128, :])
                nc.sync.dma_start(out=wkv_sb[:, k, :], in_=wkv[k * 128:(k + 1) * 128, :])
                nc.sync.dma_start(out=wo_sb[:, k, :], in_=wo[k * 128:(k + 1) * 128, :])
            nc.sync.dma_start(out=bo_sb, in_=bo_pm[:])
            nc.sync.dma_start(out=mask_sb[:, 0, :], in_=maskU[:])
            nc.sync.dma_start(out=mask_sb[:, 1, :], in_=maskU[:])
            nc.vector.memset(ones_sb, 1.0)

            for b in [bb % B for bb in range(B * REPEAT)]:
                # ---- load xT slice (feature-major, with halo) ----
                x_sb = xpool.tile([128, KT, TH], BF16, tag="x")
                nc.gpsimd.dma_start(
                    out=x_sb[:],
                    in_=xT[b].rearrange("(k p) t -> p k t", p=128),
                )

                qT = actpool.tile([128, MT, TOW], BF16, tag="qT")
                kTt = actpool.tile([128, MT, TH], BF16, tag="kT")
                v_sb = actpool.tile([128, NKW, DIM], BF16, tag="v")
                aoT = actpool.tile([128, MT, TOW], BF16, tag="aoT")

                # ---- q projection, feature-major: qT[m] = wq[:,m].T @ x ----
                for m in range(MT):
                    for c in range(2):
                        ps = pbig.tile([128, 512], F32, tag="big")
                        for k in range(KT):
                            nc.tensor.matmul(
                                ps,
                                lhsT=wq_sb[:, k, m * 128:(m + 1) * 128],
                                rhs=x_sb[:, k, W + c * 512:W + (c + 1) * 512],
                                start=(k == 0),
                                stop=(k == KT - 1),
                            )
                        nc.scalar.activation(
                            out=qT[:, m, c * 512:(c + 1) * 512], in_=ps,
                            func=AF.Copy, bias=0.0, scale=1.0,
                        )

                # ---- k projection, feature-major (incl. halo) ----
                for m in range(MT):
                    for c in range(3):
                        ps = pbig.tile([128, 512], F32, tag="big")
                        for k in range(KT):
                            nc.tensor.matmul(
                                ps[:, 0:384],
                                lhsT=wkv_sb[:, k, m * 128:(m + 1) * 128],
                                rhs=x_sb[:, k, c * 384:(c + 1) * 384],
                                start=(k == 0),
                                stop=(k == KT - 1),
                            )
                        nc.scalar.activation(
                            out=kTt[:, m, c * 384:(c + 1) * 384],
                            in_=ps[:, 0:384],
                            func=AF.Copy,
                            bias=0.0,
                            scale=1.0,
                        )

                # ---- v projection, token-major per key-window ----
                for kw in range(NKW):
                    for c in range(2):
                        ps = pbig.tile([128, 512], F32, tag="big")
                        for k in range(KT):
                            nc.tensor.matmul(
                                ps,
                                lhsT=x_sb[:, k, kw * 128:(kw + 1) * 128],
                                rhs=wkv_sb[:, k, DIM + c * 512:DIM + (c + 1) * 512],
                                start=(k == 0),
                                stop=(k == KT - 1),
                            )
                        nc.scalar.activation(
                            out=v_sb[:, kw, c * 512:(c + 1) * 512],
                            in_=ps,
                            func=AF.Copy,
                            bias=0.0,
                            scale=1.0,
                        )

                # ---- attention: key-window-major transposed scores ----
                def _emit_out_proj(c, _b=b, _qT=qT, _aoT=aoT):
                    for m in range(MT):
                        ps = pbig.tile([128, 512], F32, tag="big", name="ps_op")
                        for k in range(KT):
                            nc.tensor.matmul(
                                ps,
                                lhsT=wo_sb[:, k, m * 128:(m + 1) * 128],
                                rhs=_aoT[:, k, c * 512:(c + 1) * 512],
                                start=(k == 0),
                                stop=(k == KT - 1),
                            )
                        osb = opool.tile([128, 512], BF16, tag="outsb")
                        nc.vector.tensor_scalar_add(
                            out=osb, in0=ps, scalar1=bo_sb[:, m:m + 1]
                        )
                        nc.sync.dma_start(
                            out=outT[_b, m * 128:(m + 1) * 128,
                                     c * 512:(c + 1) * 512],
                            in_=osb,
                        )

                attn_tiles = []
                for kw in range(NKW):
                    has_a = kw >= 1          # cur-block for qw = kw-1
                    has_b = kw <= NW - 1     # prev-block for qw = kw
                    ncols = 128 * (has_a + has_b)
                    # qT is indexed by own tokens (haloed minus W)
                    qcol0 = (kw - 1) * 128 if has_a else 0
                    at = apool.tile([128, H, 256], BF16, tag="attn")
                    attn_tiles.append(at)
                    # attn tile slot permutation: slot(h) = (h%2)*8 + h//2,
                    # so same-row-group head pairs (same r, adjacent m) are
                    # adjacent slots.  Same-row-group score MMs serialize in
                    # the PE array, making same-bank drains safe.
                    for e in range(2):
                        r = e * 64
                        for mp in range(0, MT, 2):
                            sc = pbig.tile([128, 512], F32, tag="big")
                            for mi in range(2):
                                nc.tensor.matmul(
                                    sc[:, mi * 256:mi * 256 + ncols],
                                    lhsT=kTt[r:r + 64, mp + mi,
                                             kw * 128:(kw + 1) * 128],
                                    rhs=qT[r:r + 64, mp + mi,
                                           qcol0:qcol0 + ncols],
                                    start=True,
                                    stop=True,
                                    skip_group_check=True,
                                )
                            sl = e * 8 + mp
                            nc.scalar.activation(
                                out=at[:, sl:sl + 2, 0:ncols],
                                in_=sc[:].rearrange("p (h i) -> p h i", i=256)[
                                    :, :, 0:ncols
                                ],
                                func=AF.Exp,
                                bias=0.0,
                                scale=SCALE,
                            )
                            if has_a:
                                # multiplicative causal mask on the cur block:
                                # at[j, i] *= (j <= i), zeroing masked probs
                                nc.vector.tensor_mul(
                                    out=at[:, sl:sl + 2, 0:128],
                                    in0=at[:, sl:sl + 2, 0:128],
                                    in1=mask_sb,
                                )

                    # ---- pv + sums + normalize for qw = kw-1 ----
                    if kw == 0:
                        continue
                    qw = kw - 1
                    at_prev = attn_tiles[qw]       # tile kw-1: B block
                    at_cur = attn_tiles[kw]        # tile kw:   A block
                    bcol = 0 if qw == 0 else 128
                    for c in range(2):
                        pvt = ppv.tile([128, 512], F32, tag="pv")
                        srow = prows.tile([1, 1024], F32, tag="srows")
                        pv_last = None
                        srow_last = None
                        for hh in range(8):
                            h = 8 * c + hh
                            s, p = hh // 2, hh % 2
                            sl = (h % 2) * 8 + h // 2
                            hv = slice(h * 64, (h + 1) * 64)
                            pv_out = pvt[p * 64:p * 64 + 64, s * 128:(s + 1) * 128]
                            nc.tensor.matmul(
                                pv_out,
                                lhsT=v_sb[:, qw, hv],
                                rhs=at_prev[:, sl, bcol:bcol + 128],
                                start=True,
                                stop=True,
                            )
                            pv_last = nc.tensor.matmul(
                                pv_out,
                                lhsT=v_sb[:, kw, hv],
                                rhs=at_cur[:, sl, 0:128],
                                start=False,
                                stop=False,
                                skip_group_check=True,
                            )
                        srow_lasts = []
                        for p in range(2):
                            slb = p * 8 + 4 * c
                            sum_out = srow[0:1, p * 512:(p + 1) * 512]
                            nc.tensor.matmul(
                                sum_out,
                                lhsT=ones_sb[:, 0:1],
                                rhs=at_prev[:, slb:slb + 4, bcol:bcol + 128],
                                start=True,
                                stop=True,
                                skip_group_check=True,
                            )
                            srow_lasts.append(nc.tensor.matmul(
                                sum_out,
                                lhsT=ones_sb[:, 0:1],
                                rhs=at_cur[:, slb:slb + 4, 0:128],
                                start=False,
                                stop=False,
                                skip_group_check=True,
                            ))
                        rr = rpool.tile([1, 1024], F32, tag="recip")
                        ri = nc.vector.reciprocal_approx_fast(
                            out=rr[0:1, :], in_=srow[0:1, :]
                        )
                        for sv in srow_lasts:
                            tile.add_dep_helper(ri.ins, sv.ins)
                        for p in range(2):
                            rt = bpool.tile([128, 512], F32, tag="bcast")
                            nc.gpsimd.partition_broadcast(
                                rt[0:128, :],
                                rr[0:1, p * 512:(p + 1) * 512],
                                channels=128,
                            )
                            mi = nc.vector.tensor_mul(
                                out=aoT[p * 64:p * 64 + 64, 4 * c:4 * c + 4,
                                        qw * 128:(qw + 1) * 128],
                                in0=pvt[p * 64:p * 64 + 64, :].rearrange(
                                    "p (s i) -> p s i", i=128
                                ),
                                in1=rt[p * 64:p * 64 + 64, :].rearrange(
                                    "p (s i) -> p s i", i=128
                                ),
                            )
                            tile.add_dep_helper(mi.ins, pv_last.ins)

                _emit_out_proj(0)
                _emit_out_proj(1)
    nc.compile()
    return nc


_NC_CACHE = None


def _get_nc():
    global _NC_CACHE
    if _NC_CACHE is None:
        _NC_CACHE = _build_bass()
    return _NC_CACHE


def kernel(x, wq, wkv, wo, bo):
    global LAST_RESULT
    bfd = ml_dtypes.bfloat16
    x = np.asarray(x, np.float32)
    wq_b = np.asarray(wq, np.float32).astype(bfd)
    wkv_b = np.asarray(wkv, np.float32).astype(bfd)
    wo_b = np.asarray(wo, np.float32).astype(bfd)
    bo_pm = np.ascontiguousarray(
        np.asarray(bo, np.float32).reshape(MT, 128).T
    )
    # maskU[j, i] = 0 where cur-window key j > query i (causal), else 1
    maskU = np.where(
        np.arange(W)[:, None] > np.arange(W)[None, :], 0.0, 1.0
    ).astype(bfd)

    xb = x.astype(bfd)
    in_maps = []
    for c in range(NCORES):
        lo, hi = c * TOW - W, (c + 1) * TOW
        if c == 0:
            sl = np.concatenate(
                [np.zeros((B, W, DIM), bfd), xb[:, :hi]], axis=1
            )
        else:
            sl = xb[:, lo:hi]
        xT_c = np.ascontiguousarray(sl.transpose(0, 2, 1))  # [B, DIM, TH]
        in_maps.append(
            dict(xT=xT_c, wq=wq_b, wkv=wkv_b, wo=wo_b, bo_pm=bo_pm,
                 maskU=maskU)
        )

    nc = _get_nc()
    res = run_bass_kernel_spmd(
        nc, in_maps, list(range(NCORES)), trace=TRACE, **TRACE_KW
    )
    if TRACE:
        LAST_RESULT = res
    out = np.empty((B, N, DIM), np.float32)
    for c in range(NCORES):
        out[:, c * TOW:(c + 1) * TOW, :] = (
            res.results[c]["outT"].astype(np.float32).transpose(0, 2, 1)
        )
    return out
